# revision 37
# baseline (speedup 1.0000x reference)
"""Causal self-attention with LoRA (q,v) — Trainium2 Bass kernel, 8 cores.

Sharding: data-parallel over batch (B=2), tensor-parallel over heads
(16 heads -> 4 per core).  Core c handles batch c//4, heads 4*(c%4)..+4.
Each core computes its 256-dim q/k/v projection slice from the full
hidden states and its heads' full 2048x2048 causal attention locally.
No collectives; host does the (layout-only) scatter/gather.

v2 (bf16 + HAM-warm schedule):
 - All matmul operands are bf16 (host converts; rel err ~3.5e-3 vs the
   2e-2 gate).  Halves input DMA to ~5.7MB, enables FWL weight loads,
   and unlocks DVE 2x/4x modes for the causal-mask multiplies.
 - Work is emitted per t-quarter: project q/k/v for quarter t (needs
   only that quarter of x), then immediately run both head-pairs'
   attention i-block t.  The PE goes dense within ~10us of kernel
   start and stays dense, so the HAM clock-gate reaches K=8/8
   (2.4 GHz) early instead of at ~53us (v1 ran the whole projection
   phase DMA-starved at 1.2 GHz).
 - x/w DMAs are batched (512KB each) and split across the gpsimd
   (SWDGE) and sync (HWDGE) queues so the two stream in parallel.
 - Softmax normalize: reciprocal of the PSUM denominator row directly
   on DVE, broadcast across partitions via a tiny ones-outer-product
   matmul into PSUM (replaces v1's dn/rc DMA round-trip + gpsimd
   partition_broadcast, which serialized a ~9us kernel tail), then one
   DVE multiply straight out of PSUM.

Attention math is unchanged from v1: scores computed transposed
(sT[key, query]) so no on-chip transposes; softmax denominator rides
the PV matmul as a 65th lhsT column (em = exp(mask) scaling of V
folds the additive attention mask exactly); causal staircase is a
multiplicative bf16 mask after exp; the two most-diagonal 128-key
chunks per 512-query block are cropped to their upper 256 columns,
laid out so concurrently issued row-packed matmul pairs (two heads at
K=64 via PE row groups) never write the same PSUM bank.
"""

import numpy as np

B, T, DM, H = 2, 2048, 1024, 16
HD = 64          # head dim
R = 8            # LoRA rank
NCORES = 8
GPB = 4          # head-groups (cores) per batch
HPC = 4          # heads per core
DPC = HPC * HD   # 256 output dims per core
LORA_SCALE = 2.0  # lora_alpha / r
SM_SCALE = HD ** -0.5  # 0.125

KC = DM // 128   # 8 contraction chunks
MC = DPC // 128  # 2 output-dim chunks (head pairs)
TCH = T // 128   # 16 key chunks
NQ = T // 512    # 4 t-quarters == query i-blocks

_CACHE = {}


def _build_program():
    from contextlib import ExitStack

    import concourse.bass as bass
    import concourse.tile as tile
    from concourse import bacc, mybir

    f32 = mybir.dt.float32
    f32r = mybir.dt.float32r
    bf16 = mybir.dt.bfloat16
    EXP = mybir.ActivationFunctionType.Exp
    ts = bass.ts

    nc = bacc.Bacc(
        "TRN2",
        target_bir_lowering=False,
        debug=False,
        enable_asserts=True,
        num_devices=NCORES,
    )

    xT = nc.dram_tensor("xT", [DM, T], bf16, kind="ExternalInput").ap()
    wqT = nc.dram_tensor("wqT", [DM, DPC], bf16, kind="ExternalInput").ap()
    wkT = nc.dram_tensor("wkT", [DM, DPC], bf16, kind="ExternalInput").ap()
    wvT = nc.dram_tensor("wvT", [DM, DPC], bf16, kind="ExternalInput").ap()
    aq = nc.dram_tensor("aq", [R, DM], bf16, kind="ExternalInput").ap()
    av = nc.dram_tensor("av", [R, DM], bf16, kind="ExternalInput").ap()
    bqT = nc.dram_tensor("bqT", [R, DPC], bf16, kind="ExternalInput").ap()
    bvT = nc.dram_tensor("bvT", [R, DPC], bf16, kind="ExternalInput").ap()
    bvrow = nc.dram_tensor("bvrow", [1, DPC], bf16, kind="ExternalInput").ap()
    # consts: col 0-1 = biasq (per mc), 2-3 = biask, 4-19 = amask chunks
    consts = nc.dram_tensor("consts", [128, 20], f32, kind="ExternalInput").ap()
    outT = nc.dram_tensor("outT", [DPC, T], f32, kind="ExternalOutput").ap()

    with tile.TileContext(nc) as tc, ExitStack() as ctx:
        const = ctx.enter_context(tc.tile_pool(name="const", bufs=1))
        xpool = ctx.enter_context(tc.tile_pool(name="x", bufs=1))
        wpool = ctx.enter_context(tc.tile_pool(name="w", bufs=1))
        qkpool = ctx.enter_context(tc.tile_pool(name="qk", bufs=1))
        vpool = ctx.enter_context(tc.tile_pool(name="v", bufs=1))
        ppool = ctx.enter_context(tc.tile_pool(name="pT", bufs=4))
        opool = ctx.enter_context(tc.tile_pool(name="osb", bufs=6))
        psum = ctx.enter_context(tc.tile_pool(name="psum", bufs=2, space="PSUM"))
        popool = ctx.enter_context(tc.tile_pool(name="po", bufs=1, space="PSUM"))

        # ---------------- SBUF tiles ----------------
        x_all = xpool.tile([128, KC * T], bf16, tag="x")
        wq_raw = wpool.tile([128, KC * DPC], bf16, tag="wqr")
        wv_raw = wpool.tile([128, KC * DPC], bf16, tag="wvr")
        wq_f = wpool.tile([128, KC * DPC], bf16, tag="wqf")
        wk_f = wpool.tile([128, KC * DPC], bf16, tag="wkf")
        wv_f = wpool.tile([128, KC * DPC], bf16, tag="wvf")
        aq_sb = const.tile([R, DM], bf16, tag="aq")
        av_sb = const.tile([R, DM], bf16, tag="av")
        bqT_sb = const.tile([R, DPC], bf16, tag="bqT")
        bvT_sb = const.tile([R, DPC], bf16, tag="bvT")
        bvrow_sb = const.tile([1, DPC], bf16, tag="bvrow")
        consts_sb = const.tile([128, 20], f32, tag="consts")
        em = const.tile([128, TCH], bf16, tag="em")
        stair = const.tile([128, 896], bf16, tag="stair")
        ones_bf = const.tile([1, 128], bf16, tag="ones_bf")
        # ones2[k, m] = 1 if (k == 0) == (m < 64): K=2 selector so one
        # matmul broadcasts rc0 to partitions 0-63 and rc1 to 64-127
        ones2_f = const.tile([2, 128], f32, tag="ones2_f")
        ones2 = const.tile([2, 128], f32r, tag="ones2")

        qT_sb = [qkpool.tile([128, T], bf16, tag=f"qT{mc}", name=f"qT{mc}") for mc in range(MC)]
        kT_sb = [qkpool.tile([128, T], bf16, tag=f"kT{mc}", name=f"kT{mc}") for mc in range(MC)]
        v2_sb = [vpool.tile([128, HPC * (HD + 1)], bf16, tag=f"v2{j}", name=f"v2{j}") for j in range(TCH)]

        # ---------------- DMAs ----------------
        # gpsimd (SWDGE) queue: LoRA smalls, then the low kc-half of each
        # x quarter.  sync (HWDGE) queue: consts, weights, the high
        # kc-half of each x quarter.  Both stream concurrently.
        xsrc = xT.rearrange("(g p) t -> p g t", p=128)
        xdst = x_all[:].rearrange("p (g t) -> p g t", g=KC)

        def wsrc(w):
            return w.rearrange("(g p) d -> p g d", p=128)

        def wdst(t):
            return t[:].rearrange("p (g d) -> p g d", g=KC)

        nc.gpsimd.dma_start(aq_sb[:], aq)
        nc.gpsimd.dma_start(av_sb[:], av)
        nc.gpsimd.dma_start(bvrow_sb[:], bvrow)
        nc.gpsimd.dma_start(xdst[:, 0:4, ts(0, 512)], xsrc[:, 0:4, ts(0, 512)])
        nc.gpsimd.dma_start(wdst(wv_raw), wsrc(wvT))
        for tq in range(1, NQ):
            nc.gpsimd.dma_start(xdst[:, 0:4, ts(tq, 512)], xsrc[:, 0:4, ts(tq, 512)])

        nc.sync.dma_start(consts_sb[:], consts)
        nc.sync.dma_start(bqT_sb[:], bqT)
        nc.sync.dma_start(bvT_sb[:], bvT)
        nc.sync.dma_start(wdst(wq_raw), wsrc(wqT))
        nc.sync.dma_start(xdst[:, 4:8, ts(0, 512)], xsrc[:, 4:8, ts(0, 512)])
        nc.sync.dma_start(wdst(wk_f), wsrc(wkT))
        for tq in range(1, NQ):
            nc.sync.dma_start(xdst[:, 4:8, ts(tq, 512)], xsrc[:, 4:8, ts(tq, 512)])

        # ---------------- small setup compute ----------------
        nc.vector.memset(ones_bf[:], 1.0)
        # ones2_f[k, m] = 1 iff 0 <= m - 64k < 64
        nc.vector.memset(ones2_f[:], 1.0)
        nc.gpsimd.affine_select(
            out=ones2_f[:], in_=ones2_f[:],
            compare_op=mybir.AluOpType.is_ge,
            fill=0.0, base=0, pattern=[[1, 128]], channel_multiplier=-64,
        )
        nc.gpsimd.affine_select(
            out=ones2_f[:], in_=ones2_f[:],
            compare_op=mybir.AluOpType.is_ge,
            fill=0.0, base=63, pattern=[[-1, 128]], channel_multiplier=64,
        )
        nc.vector.tensor_copy(ones2[:], ones2_f[:])
        # em[p, jb] = exp(amask[128*jb + p])
        nc.scalar.activation(em[:], consts_sb[:, 4:20], EXP)
        # causal staircase: stair[p, m] = 1.0 if m >= p + 384 else 0.0
        nc.gpsimd.memset(stair[:], 1.0)
        nc.gpsimd.affine_select(
            out=stair[:],
            in_=stair[:],
            compare_op=mybir.AluOpType.is_ge,
            fill=0.0,
            base=-384,
            pattern=[[1, 896]],
            channel_multiplier=-1,
        )

        def stair_slice(d, w):
            # full-width chunk (w=512): mask[p, f] = f >= p + 128 d
            # cropped chunk  (w=256, f' = f-256): f' >= p + 128 d - 256
            start = 384 - 128 * d if w == 512 else 640 - 128 * d
            return stair[:, start : start + w]

        # ---------------- LoRA fold: W' = W + A.T @ (2 B.T) ----------------
        # 4 kc-chunks per [128, 1024] psum tile (each 256-wide matmul dst
        # stays within a bank), one wide DVE add per tile.
        def fold(raw, a_sb, bT_sb, dst):
            for half in range(2):
                dps = psum.tile([128, 4 * DPC], f32, tag="sc")
                for j in range(4):
                    kc = 4 * half + j
                    nc.tensor.matmul(
                        dps[:, ts(j, DPC)], a_sb[:, ts(kc, 128)], bT_sb[:],
                        start=True, stop=True,
                    )
                nc.vector.tensor_add(
                    dst[:, half * 4 * DPC : (half + 1) * 4 * DPC],
                    raw[:, half * 4 * DPC : (half + 1) * 4 * DPC],
                    dps[:],
                )

        # ---------------- projections ----------------
        def project_qk(w_f, dst, bias_col, mc, nb):
            ps = psum.tile([128, 512], f32, tag="sc")
            for kc in range(KC):
                nc.tensor.matmul(
                    ps[:],
                    w_f[:, kc * DPC + mc * 128 : kc * DPC + (mc + 1) * 128],
                    x_all[:, kc * T + nb * 512 : kc * T + (nb + 1) * 512],
                    start=(kc == 0),
                    stop=(kc == KC - 1),
                )
            nc.vector.tensor_add(
                dst[:, ts(nb, 512)],
                ps[:],
                consts_sb[:, bias_col : bias_col + 1].to_broadcast((128, 512)),
            )

        # v in natural [t, d] orientation, em-scaled, with the denominator
        # (em) column appended per head: [128, 4*65].
        def project_v(jb):
            ps = psum.tile([128, DPC], f32, tag="sc")
            for kc in range(KC):
                nc.tensor.matmul(
                    ps[:],
                    x_all[:, kc * T + jb * 128 : kc * T + (jb + 1) * 128],
                    wv_f[:, ts(kc, DPC)],
                    start=(kc == 0),
                    stop=False,
                )
            nc.tensor.matmul(  # + ones(t) x bias_v
                ps[:], ones_bf[:], bvrow_sb[:], start=False, stop=True
            )
            v2 = v2_sb[jb]
            em_col = em[:, jb : jb + 1]
            for hl in range(HPC):
                nc.vector.tensor_mul(
                    v2[:, hl * (HD + 1) : hl * (HD + 1) + HD],
                    ps[:, ts(hl, HD)],
                    em_col.to_broadcast((128, HD)),
                )
            nc.vector.tensor_copy(
                v2[:, HD : HPC * (HD + 1) : HD + 1],
                em_col.to_broadcast((128, HPC)),
            )

        # ---------------- attention for one head pair, one i-block ----------------
        blocksA = []  # blocks awaiting normalize stage A
        blocksB = []  # blocks awaiting normalize stage B

        def attention_ib(pr, ib):
            qT, kT = qT_sb[pr], kT_sb[pr]
            nch = 4 * ib + 4  # causal key chunks per head
            fulls, crops = [], []
            for jb in range(nch):
                d = jb - 4 * ib
                if d >= 2:
                    for hl in (0, 1):
                        crops.append((hl, jb, 256, 256, d))
                else:
                    for hl in (0, 1):
                        fulls.append((hl, jb, 0, 512, d))
            # groups: list of (chunk, col_off). Fulls: one (h, h') jb pair
            # per [128, 1024] group, cols 0/512.  Crops: four 256-wide
            # diagonal chunks in one group, bank-disjoint per row-packed
            # concurrent (h, h') pair.
            groups = []
            for i in range(0, len(fulls), 2):
                groups.append([(c, j * 512) for j, c in enumerate(fulls[i : i + 2])])
            if crops:
                groups.append(
                    [(crops[0], 0), (crops[1], 512), (crops[2], 256), (crops[3], 768)]
                )

            po = [
                popool.tile([65, 512], f32, tag=f"po{hl}", name=f"po{pr}_{ib}_{hl}")
                for hl in (0, 1)
            ]
            for g in groups:
                width = sum(c[3] for c, _ in g)
                ps = psum.tile([128, width], f32, tag="sc")
                for (hl, jb, qo, w, d), off in g:
                    nc.tensor.matmul(
                        ps[:, off : off + w],
                        kT[ts(hl, 64), ts(jb, 128)],
                        qT[ts(hl, 64), ib * 512 + qo : ib * 512 + qo + w],
                        start=True,
                        stop=True,
                    )
                pT = ppool.tile([128, width], bf16, tag="pT")
                nc.scalar.activation(pT[:], ps[:], EXP, scale=SM_SCALE)
                # causal staircase on partial chunks; merge the (h, h')
                # twin segments into one 3-D op when adjacent
                i = 0
                while i < len(g):
                    (hl, jb, qo, w, d), off_i = g[i]
                    if d < 0:
                        i += 1
                        continue
                    msk = stair_slice(d, w)
                    twin = (
                        i + 1 < len(g)
                        and g[i + 1][0][1] == jb
                        and g[i + 1][0][3] == w
                        and g[i + 1][1] == off_i + w
                    )
                    if twin:
                        seg = pT[:, off_i : off_i + 2 * w].rearrange(
                            "p (two n) -> p two n", two=2
                        )
                        nc.vector.tensor_mul(
                            seg,
                            seg,
                            msk.unsqueeze(1).broadcast_to((128, 2, w)),
                        )
                        i += 2
                    else:
                        nc.vector.tensor_mul(
                            pT[:, off_i : off_i + w],
                            pT[:, off_i : off_i + w],
                            msk,
                        )
                        i += 1
                # PV: outT[d, i] accumulation per head; denominator column
                # (em) rides along as lhsT column 64.
                for (hl, jb, qo, w, d), off in g:
                    nc.tensor.matmul(
                        po[hl][:, qo : qo + w],
                        v2_sb[jb][:, (2 * pr + hl) * (HD + 1) : (2 * pr + hl + 1) * (HD + 1)],
                        pT[:, off : off + w],
                        start=(jb == 0),
                        stop=(jb == nch - 1),
                    )
            # normalize: out[:64] / denom (row 64) per column.  Reciprocal
            # of the denominator row on DVE, partition-broadcast via a
            # K=1 ones matmul into PSUM, one DVE multiply out of PSUM.
            # The sbp copy alone releases po, so the next i-block's PV is
            # not gated on the normalize chain.  The chain itself is
            # split into two stages emitted one and two blocks later, so
            # its cross-engine latency always hides behind real work and
            # never head-of-line-blocks the DVE FIFO.
            sbps = []
            for hl in (0, 1):
                sbp = opool.tile([65, 512], f32, tag="sbp")
                nc.vector.tensor_copy(sbp[:], po[hl][:])
                sbps.append(sbp)
            blocksA.append((sbps, pr, ib))

        # normalize: out[:64] / denom (row 64) per column.  DVE
        # reciprocal is free-size-bound (~6.5 cyc/elem; [1, 512] would
        # cost 3.3us), so stage A reshapes the denominator row onto 128
        # partitions (SBUF->SBUF DMA), recips on [128, 4] (~0.2us),
        # scatters back, and partition-broadcasts both heads via two
        # K=1 ones matmuls into one [128, 512] PSUM tile.  Stage B (two
        # blocks later, when rb2 is long ready) does one DVE multiply
        # per head and stores.
        def norm_stage_a(direct=False):
            if not blocksA:
                return
            sbps, pr, ib = blocksA.pop(0)
            if direct:
                # kernel-tail variant: the [1, 512] DVE reciprocal costs
                # 3.3us of (then-idle) DVE but has no DMA round-trip
                # latency, shortening the exposed serial tail.
                rbs = []
                for hl in (0, 1):
                    rcd = opool.tile([1, 512], f32r, tag="rcd")
                    with nc.allow_low_precision(reason="softmax denom reciprocal; f32r feeds the broadcast matmul"):
                        nc.vector.reciprocal(rcd[:], sbps[hl][64:65, :])
                    rb_h = psum.tile([64, 512], f32, tag="rb", bufs=2)
                    nc.tensor.matmul(rb_h[:], ones2[0:1, 0:64], rcd[:], start=True, stop=True)
                    rbs.append(rb_h)
                blocksB.append((sbps, rbs, pr, ib))
                return
            rc2 = opool.tile([2, 512], f32r, tag="rc2")
            for hl in (0, 1):
                dn = opool.tile([128, 4], f32, tag="dn")
                nc.gpsimd.dma_start(
                    dn[:], sbps[hl][64:65, :].rearrange("o (p c) -> o p c", p=128)
                )
                dnr = opool.tile([128, 4], f32r, tag="dnr")
                with nc.allow_low_precision(reason="softmax denom reciprocal; f32r feeds the broadcast matmul"):
                    nc.vector.reciprocal(dnr[:], dn[:])
                nc.sync.dma_start(
                    rc2[hl : hl + 1, :].rearrange("o (p c) -> o p c", p=128), dnr[:]
                )
            rb2 = psum.tile([128, 512], f32, tag="rb", bufs=2)
            nc.tensor.matmul(rb2[:], ones2[:], rc2[:], start=True, stop=True)
            blocksB.append((sbps, rb2, pr, ib))

        def norm_stage_b():
            if not blocksB:
                return
            sbps, rb2, pr, ib = blocksB.pop(0)
            for hl in (0, 1):
                rbv = rb2[hl][:] if isinstance(rb2, list) else rb2[ts(hl, 64), :]
                oT = opool.tile([64, 512], f32, tag="oT")
                nc.vector.tensor_mul(oT[:], sbps[hl][0:64, :], rbv)
                nc.sync.dma_start(
                    outT[(2 * pr + hl) * HD : (2 * pr + hl + 1) * HD, ts(ib, 512)],
                    oT[:],
                )

        # ---------------- emission schedule ----------------
        # Warmup: dummy matmuls (inputs are memset tiles, no DMA deps)
        # keep the PE busy from dispatch (~10us) while the first x/w
        # DMAs land, so the HAM clock-gate reaches K=8/8 before the
        # real work starts instead of ~20us into it.
        dummy_row = const.tile([1, 512], bf16, tag="dummy_row")
        nc.vector.memset(dummy_row[:], 0.5)
        wps = popool.tile([65, 512], f32, tag="po0", name="warm")
        for wi in range(16):
            nc.tensor.matmul(wps[:], ones_bf[:, 0:65], dummy_row[:], start=True, stop=True)

        # Per t-quarter: project q/k/v for that quarter (both head
        # pairs), then run both pairs' attention i-block.  Keeps the PE
        # dense from ~10us on (HAM warm) and spreads ACT exp work evenly.
        fold(wq_raw, aq_sb, bqT_sb, wq_f)
        fold(wv_raw, av_sb, bvT_sb, wv_f)
        for tq in range(NQ):
            for mc in range(MC):
                project_qk(wq_f, qT_sb[mc], mc, mc, tq)
                project_qk(wk_f, kT_sb[mc], 2 + mc, mc, tq)
            for jb in range(4 * tq, 4 * tq + 4):
                project_v(jb)
            for pr in range(2):
                attention_ib(pr, tq)
                # stage A for the previous block, stage B for the one
                # before that — each stage's inputs are then already a
                # full block old, so nothing waits.
                while len(blocksA) > 1:
                    norm_stage_a()
                while len(blocksB) > 1:
                    norm_stage_b()
        # drain the normalize pipeline (last two blocks)
        norm_stage_a(direct=True)
        norm_stage_b()
        norm_stage_b()

    nc.compile()
    return nc


def _shard_inputs(inputs):
    """Full inputs -> per-core input maps (host-side layout work only)."""
    import ml_dtypes

    bf = ml_dtypes.bfloat16
    hs = np.asarray(inputs["hidden_states"], dtype=np.float32)
    am = np.asarray(inputs["attention_mask"], dtype=np.float32)
    Wq = np.asarray(inputs["Wq"], dtype=np.float32)
    Wk = np.asarray(inputs["Wk"], dtype=np.float32)
    Wv = np.asarray(inputs["Wv"], dtype=np.float32)
    bq = np.asarray(inputs["bq"], dtype=np.float32)
    bk = np.asarray(inputs["bk"], dtype=np.float32)
    bv = np.asarray(inputs["bv"], dtype=np.float32)
    Aq = np.asarray(inputs["Aq"], dtype=np.float32)
    Bq = np.asarray(inputs["Bq"], dtype=np.float32)
    Av = np.asarray(inputs["Av"], dtype=np.float32)
    Bv = np.asarray(inputs["Bv"], dtype=np.float32)

    def c(x):
        return np.ascontiguousarray(x)

    def cb(x):
        return np.ascontiguousarray(x).astype(bf)

    xTs = [cb(hs[b].T) for b in range(B)]
    aq_b, av_b = cb(Aq), cb(Av)
    in_maps = []
    for core in range(NCORES):
        b, g = core // GPB, core % GPB
        sl = slice(g * DPC, (g + 1) * DPC)
        consts = np.zeros((128, 20), np.float32)
        consts[:, 0] = bq[sl][:128]
        consts[:, 1] = bq[sl][128:]
        consts[:, 2] = bk[sl][:128]
        consts[:, 3] = bk[sl][128:]
        consts[:, 4:20] = am[b, 0, 0, :].reshape(TCH, 128).T
        in_maps.append(
            {
                "xT": xTs[b],
                "wqT": cb(Wq[sl].T),
                "wkT": cb(Wk[sl].T),
                "wvT": cb(Wv[sl].T),
                "aq": aq_b,
                "av": av_b,
                "bqT": cb(LORA_SCALE * Bq[sl].T),
                "bvT": cb(LORA_SCALE * Bv[sl].T),
                "bvrow": cb(bv[sl].reshape(1, DPC)),
                "consts": consts,
            }
        )
    return in_maps


def _run(inputs, trace=False):
    from concourse.bass_utils import run_bass_kernel_spmd

    if "nc" not in _CACHE:
        _CACHE["nc"] = _build_program()
    nc = _CACHE["nc"]
    in_maps = _shard_inputs(inputs)
    res = run_bass_kernel_spmd(nc, in_maps, list(range(NCORES)), trace=trace)
    out = np.empty((B, T, DM), dtype=np.float32)
    for core in range(NCORES):
        b, g = core // GPB, core % GPB
        out[b, :, g * DPC : (g + 1) * DPC] = res.results[core]["outT"].T
    return out, res


def kernel(**inputs) -> np.ndarray:
    out, _ = _run(inputs, trace=False)
    return out
